# revision 26
# baseline (speedup 1.0000x reference)
"""Bahdanau attention (optimized) Trainium2 kernel.

Problem (hardcoded shapes): B=32, S=2048, ENC=DEC=1024.
    dec_proj = dec_hidden @ W_dec + b                  [B, DEC]
    projT    = W_enc.T @ enc_outputs[b].T              [DEC, S]  (per batch)
    energyT  = tanh(projT + dec_proj[b][:, None])      [DEC, S]
    scores   = v.T @ energyT (+ mask)                  [S]
    attn     = softmax(scores)                         (scores in ~[-4, 4]: plain exp is safe)
    context  = attn @ enc_outputs[b]                   [ENC]

Sharding: data-parallel over batch, 4 batches per core on 8 NeuronCores.
Each core holds the full (small) weights.

Host-side prep (layout only): enc is fed twice in bf16 — once pre-transposed
to [DEC-on-partitions] for the projection matmul (the PE contracts along
partitions, and the natural layout has S on partitions), and once in natural
layout for the context matmul.  All matmuls run in bf16 with fp32 PSUM
accumulation (measured end-to-end rel. error ~3.3e-3); softmax bookkeeping
stays fp32.

Per 512-wide sequence block: 64 projection matmuls (N=512, stationary=W tile)
stream back-to-back at the PE N=512 roofline (~216ns spacing); tanh with
per-partition bias on ACT; the 8 scores matmuls (stationary = 1-column v) are
batched per block; scores row -> columns via tiny K=1 float32r matmuls; exp
on ACT with additive-mask bias.  The context accumulation runs on the
otherwise-idle DVE as a fused scalar_tensor_tensor chain (acc += en * w per
128-chunk) with a final PE partition-reduce; the last block of the last
batch goes through the PE so the serial DVE chain never becomes the kernel
tail.  Block tails and batch epilogues are software-pipelined into the next
block's projection j-loop so the PE stream stays gapless; weights and the
first enc tiles arrive as k-sliced DMAs spread over the three DMA rings
(scalar/sync HWDGE + gpsimd SWDGE) to cut the startup ramp.
"""

import os
import sys

for _p in ("/opt/trn_rl_repo",):
    if os.path.isdir(_p) and _p not in sys.path:
        sys.path.append(_p)

import numpy as np
import ml_dtypes

import concourse.bass as bass
import concourse.mybir as mybir
import concourse.tile as tile
from concourse import bacc
from concourse.bass_utils import run_bass_kernel_spmd
from concourse.masks import make_identity

BF16 = mybir.dt.bfloat16
F32 = mybir.dt.float32
F32R = mybir.dt.float32r

N_CORES = 8
B, S, E, D = 32, 2048, 1024, 1024
NB = B // N_CORES          # batches per core = 4
SB = 512                   # sequence block
NBLK = S // SB             # 4 blocks per batch
NCH = SB // 128            # 4 chunks of 128 per block
NCHB = S // 128            # 16 chunks per batch
KT = E // 128              # 8 contraction tiles
DT = D // 128              # 8 d tiles

_COMPILED = None


def _build():
    nc = bacc.Bacc(None, target_bir_lowering=False, debug=False)

    encT_bf = nc.dram_tensor("encT_bf", [NB, E, S], BF16, kind="ExternalInput").ap()
    enc_bf = nc.dram_tensor("enc_bf", [NB, S, E], BF16, kind="ExternalInput").ap()
    w_enc = nc.dram_tensor("w_enc", [E, D], BF16, kind="ExternalInput").ap()
    w_dec = nc.dram_tensor("w_dec", [D, D], BF16, kind="ExternalInput").ap()
    dec_hT = nc.dram_tensor("dec_hT", [128, KT, NB], BF16, kind="ExternalInput").ap()
    attn_b = nc.dram_tensor("attn_b", [128, DT], F32, kind="ExternalInput").ap()
    v_in = nc.dram_tensor("v_in", [128, DT], BF16, kind="ExternalInput").ap()
    maskT = nc.dram_tensor("maskT", [128, NB * NCHB], F32, kind="ExternalInput").ap()

    out_ctx = nc.dram_tensor("out_ctx", [NB, E], F32, kind="ExternalOutput").ap()
    out_attn = nc.dram_tensor("out_attn", [NB, S], F32, kind="ExternalOutput").ap()

    with tile.TileContext(nc) as tc:
        with (
            tc.tile_pool(name="const", bufs=1) as const_pool,
            tc.tile_pool(name="et", bufs=2) as et_pool,
            tc.tile_pool(name="en", bufs=2) as en_pool,
            tc.tile_pool(name="energy", bufs=2) as energy_pool,
            tc.tile_pool(name="expw", bufs=2) as expw_pool,
            tc.tile_pool(name="acc", bufs=2) as acc_pool,
            tc.tile_pool(name="small", bufs=4) as small_pool,
            tc.tile_pool(name="pproj", bufs=2, space="PSUM") as proj_psum,
            tc.tile_pool(name="psmall", bufs=2, space="PSUM") as small_psum,
            tc.tile_pool(name="pctxr", bufs=2, space="PSUM") as ctxr_psum,
        ):
            # ---- resident constants -------------------------------------
            # Queue split: the critical first bytes (w_enc on scalar, et0 on
            # sync) get their rings to themselves; w_dec streams through
            # SWDGE as 8 k-slices so the interleaved dec_proj matmuls can
            # start on slice 0.  Tiny consts go first (KBs).
            dech_sb = const_pool.tile([128, KT, NB], BF16)
            nc.scalar.dma_start(dech_sb[:], dec_hT[:])
            attnb_sb = const_pool.tile([128, DT], F32)
            nc.scalar.dma_start(attnb_sb[:], attn_b[:])
            v_sb = const_pool.tile([128, DT], BF16)
            nc.scalar.dma_start(v_sb[:], v_in[:])
            mask_sb = const_pool.tile([128, NB * NCHB], F32)
            nc.scalar.dma_start(mask_sb[:], maskT[:])
            w_enc_sb = const_pool.tile([128, KT, D], BF16)
            w_enc_r = w_enc.rearrange("(k p) d -> p k d", p=128)
            for k in range(KT):
                nc.scalar.dma_start(w_enc_sb[:, k, :], w_enc_r[:, k, :])
            w_dec_sb = const_pool.tile([128, KT, D], BF16)
            w_dec_r = w_dec.rearrange("(k p) d -> p k d", p=128)
            for k in range(KT):
                nc.gpsimd.dma_start(w_dec_sb[:, k, :], w_dec_r[:, k, :])

            one_11 = const_pool.tile([1, 1], F32)
            nc.gpsimd.memset(one_11[:], 1.0)
            one_2f = const_pool.tile([1, 2], F32)
            nc.gpsimd.memset(one_2f[:], 1.0)
            one_2r = const_pool.tile([1, 2], F32R)
            nc.vector.tensor_copy(one_2r[:], one_2f[:])
            ones_row = const_pool.tile([1, 128], F32)
            nc.gpsimd.memset(ones_row[:], 1.0)
            ones_2 = const_pool.tile([128, 2], F32)
            nc.gpsimd.memset(ones_2[:], 1.0)
            ones_col_bf = const_pool.tile([128, 1], BF16)
            nc.gpsimd.memset(ones_col_bf[:], 1.0)
            id_sb = const_pool.tile([128, 128], BF16)
            make_identity(nc, id_sb[:])

            # ---- dec_projT[d, b] = (dec_hidden @ W_dec + b).T ------------
            # Computed as dec_proj rows [NB, D] with wide (N=512) matmuls,
            # then 8 tiny PE transposes to columns.  Emitted inside the first
            # block (after proj group j=0) so the first projection matmuls
            # don't wait for w_dec: tanh j stalls briefly, which the
            # block-end scores batch absorbs.
            dp_sb = const_pool.tile([128, DT, NB], F32)

            def emit_dec_proj():
                dpp = ctxr_psum.tile([NB, D], F32, tag="pcr")
                for k in range(KT):
                    for h in range(2):
                        nc.tensor.matmul(
                            dpp[:, h * 512:(h + 1) * 512],
                            dech_sb[:, k, :],
                            w_dec_sb[:, k, h * 512:(h + 1) * 512],
                            start=(k == 0),
                            stop=(k == KT - 1),
                            skip_group_check=True,
                        )
                dpr = small_pool.tile([NB, D], BF16, tag="dpr")
                nc.scalar.activation(
                    dpr[:], dpp[:], mybir.ActivationFunctionType.Copy
                )
                for j in range(DT):
                    pj = small_psum.tile([128, NB], BF16, tag="sp")
                    nc.tensor.transpose(
                        pj[:], dpr[:, j * 128:(j + 1) * 128], id_sb[:NB, :NB]
                    )
                    nc.scalar.activation(
                        dp_sb[:, j, :],
                        pj[:],
                        mybir.ActivationFunctionType.Identity,
                        bias=attnb_sb[:, j:j + 1],
                    )

            emit_dec_proj()

            # ---- main loop ----------------------------------------------
            # Software pipelining: each block's tail (scores matmuls, the
            # scores-row transpose, exp, and the context accumulation) is
            # emitted inside the NEXT block's projection j-loop, so the PE
            # never waits on the ACT/DVE hops in the tail chain.  Batch
            # epilogues are likewise deferred into the next batch's second
            # block.
            pending_epilogue = None
            pending_tail = None
            for b in range(NB):
                expw_all = expw_pool.tile([128, NCHB], F32)
                state = {"acc": None, "pcr": None, "sc": None}

                for blk in range(NBLK):
                    t0 = blk * NCH
                    et = et_pool.tile([128, KT, SB], BF16)
                    et_src = encT_bf[b].rearrange("(k p) s -> p k s", p=128)[
                        :, :, blk * SB:(blk + 1) * SB
                    ]
                    nc.sync.dma_start(et[:, :KT // 2, :], et_src[:, :KT // 2, :])
                    nc.sync.dma_start(et[:, KT // 2:, :], et_src[:, KT // 2:, :])
                    en = en_pool.tile([128, NCH, E], BF16)
                    nc.scalar.dma_start(
                        en[:],
                        enc_bf[b, blk * SB:(blk + 1) * SB, :].rearrange(
                            "(c p) e -> p c e", p=128
                        ),
                    )

                    egy = energy_pool.tile([128, DT, SB], BF16)
                    for j in range(DT):
                        pp = proj_psum.tile([128, SB], F32)
                        for k in range(KT):
                            nc.tensor.matmul(
                                pp[:],
                                w_enc_sb[:, k, j * 128:(j + 1) * 128],
                                et[:, k, :],
                                start=(k == 0),
                                stop=(k == KT - 1),
                            )
                        if pending_tail is not None:
                            if j == 0:
                                pending_tail[0]()
                            elif j == 1:
                                pending_tail[1]()
                                pending_tail = None
                        if blk == 1 and j == 3 and pending_epilogue is not None:
                            pending_epilogue()
                            pending_epilogue = None
                        nc.scalar.activation(
                            egy[:, j, :],
                            pp[:],
                            mybir.ActivationFunctionType.Tanh,
                            bias=dp_sb[:, j, b:b + 1],
                        )

                    def tail1(b=b, egy=egy, state=state):
                        # scores: batched v-matmuls (stationary = 1-col v)
                        psc = small_psum.tile([1, SB], F32, tag="sp")
                        for j in range(DT):
                            nc.tensor.matmul(
                                psc[:],
                                v_sb[:, j:j + 1],
                                egy[:, j, :],
                                start=(j == 0),
                                stop=(j == DT - 1),
                                skip_group_check=True,
                            )
                        sc_sb = small_pool.tile([1, SB], F32R, tag="scsb")
                        nc.vector.tensor_copy(sc_sb[:], psc[:])
                        state["sc"] = sc_sb

                    def tail2(b=b, blk=blk, t0=t0, en=en, state=state,
                              expw_all=expw_all):
                        sc_sb = state["sc"]
                        # scores row -> 4 columns of 128 (K=1 f32r matmuls)
                        pt = small_psum.tile([128, 2 * NCH], F32, tag="sp")
                        for c in range(NCH):
                            nc.tensor.matmul(
                                pt[:, 2 * c:2 * c + 2],
                                sc_sb[:, c * 128:(c + 1) * 128],
                                one_2r[:],
                                start=True,
                                stop=True,
                                skip_group_check=True,
                            )
                        for c in range(NCH):
                            t = t0 + c
                            nc.scalar.activation(
                                expw_all[:, t:t + 1],
                                pt[:, 2 * c:2 * c + 1],
                                mybir.ActivationFunctionType.Exp,
                                bias=mask_sb[:, b * NCHB + t:b * NCHB + t + 1],
                            )
                        # context: blocks 0..2 (and all blocks of non-final
                        # batches) accumulate on the otherwise-idle DVE:
                        #   acc[p,e] (+)= en[p,e] * expw[p]  per chunk.
                        # The final block of the final batch goes through the
                        # PE instead (the serial DVE chain would be the
                        # kernel tail), merged with the acc partition-reduce.
                        if blk < NBLK - 1 or b < NB - 1:
                            acc = state["acc"]
                            for c in range(NCH):
                                t = t0 + c
                                last = t == (
                                    4 * NCH - NCH - 1 if b == NB - 1 else NCHB - 1
                                )
                                nacc = acc_pool.tile(
                                    [128, E], BF16 if last else F32,
                                    tag="accb" if last else "acc",
                                )
                                if acc is None:
                                    nc.vector.tensor_scalar_mul(
                                        nacc[:], en[:, c, :], expw_all[:, t:t + 1]
                                    )
                                else:
                                    nc.vector.scalar_tensor_tensor(
                                        nacc[:], en[:, c, :],
                                        expw_all[:, t:t + 1], acc[:],
                                        mybir.AluOpType.mult,
                                        mybir.AluOpType.add,
                                    )
                                acc = nacc
                            state["acc"] = acc
                        else:
                            acc = state["acc"]
                            pcr = ctxr_psum.tile([1, E], F32, tag="pcr")
                            for h in range(2):
                                nc.tensor.matmul(
                                    pcr[:, h * 512:(h + 1) * 512],
                                    ones_col_bf[:],
                                    acc[:, h * 512:(h + 1) * 512],
                                    start=True,
                                    stop=False,
                                    skip_group_check=True,
                                )
                            ewb = small_pool.tile([128, NCH], BF16, tag="ewb")
                            nc.vector.tensor_copy(
                                ewb[:], expw_all[:, t0:t0 + NCH]
                            )
                            for c in range(NCH):
                                for h in range(2):
                                    nc.tensor.matmul(
                                        pcr[:, h * 512:(h + 1) * 512],
                                        ewb[:, c:c + 1],
                                        en[:, c, h * 512:(h + 1) * 512],
                                        start=False,
                                        stop=(c == NCH - 1),
                                        skip_group_check=True,
                                    )
                            state["pcr"] = pcr

                    if b == NB - 1 and blk == NBLK - 1:
                        tail1()
                        tail2()
                    else:
                        pending_tail = (tail1, tail2)

                # ---- batch epilogue (deferred into the next batch) ------
                def make_epilogue(b=b, state=state, expw_all=expw_all):
                    def attn_part():
                        # Z = sum(expw): free-reduce on DVE, partition-reduce
                        # + broadcast via tiny in-stream PE matmuls (gpsimd
                        # custom ops would thrash Q7 libraries, ~7us each).
                        zc = small_pool.tile([128, 1], F32, tag="zc")
                        nc.vector.tensor_reduce(
                            zc[:], expw_all[:], mybir.AxisListType.X,
                            mybir.AluOpType.add,
                        )
                        pzz = small_psum.tile([1, 2], F32, tag="sp")
                        nc.tensor.matmul(pzz[:], zc[:], ones_2[:],
                                         start=True, stop=True,
                                         skip_group_check=True)
                        z_sb = small_pool.tile([1, 1], F32, tag="z")
                        nc.vector.tensor_copy(z_sb[:], pzz[:, 0:1])
                        rz = small_pool.tile([1, 1], F32, tag="rz")
                        nc.vector.reciprocal(rz[:], z_sb[:])
                        prz = small_psum.tile([128, 1], F32, tag="sp")
                        nc.tensor.matmul(prz[:], ones_row[:], rz[:],
                                         start=True, stop=True,
                                         skip_group_check=True)
                        rz_col = small_pool.tile([128, 1], F32, tag="rzc")
                        nc.vector.tensor_copy(rz_col[:], prz[:])

                        attn_fin = small_pool.tile([128, NCHB], BF16, tag="af")
                        nc.scalar.activation(
                            attn_fin[:],
                            expw_all[:],
                            mybir.ActivationFunctionType.Copy,
                            scale=rz_col[:],
                        )
                        pat = small_psum.tile([16, 128], BF16, tag="sp")
                        nc.tensor.transpose(pat[:], attn_fin[:], id_sb[:])
                        at_sb = small_pool.tile([16, 128], F32, tag="atsb")
                        nc.vector.tensor_copy(at_sb[:], pat[:])
                        nc.sync.dma_start(
                            out_attn[b].rearrange("(t s) -> t s", s=128),
                            at_sb[:],
                        )
                        return rz

                    def ctx_part(rz):
                        if state["pcr"] is None:
                            acc = state["acc"]
                            pcr_l = ctxr_psum.tile([1, E], F32, tag="pcr")
                            for h in range(2):
                                nc.tensor.matmul(
                                    pcr_l[:, h * 512:(h + 1) * 512],
                                    ones_col_bf[:],
                                    acc[:, h * 512:(h + 1) * 512],
                                    start=True,
                                    stop=True,
                                    skip_group_check=True,
                                )
                        else:
                            pcr_l = state["pcr"]
                        ctx_sb = small_pool.tile([1, E], F32, tag="ctx")
                        nc.scalar.activation(
                            ctx_sb[:],
                            pcr_l[:],
                            mybir.ActivationFunctionType.Copy,
                            scale=rz[:],
                        )
                        nc.scalar.dma_start(out_ctx[b], ctx_sb[:])

                    def epi():
                        ctx_part(attn_part())
                    return epi, attn_part, ctx_part

                epi, attn_part, ctx_part = make_epilogue()
                if b == NB - 1:
                    # Final batch: tails were emitted inline; finish now.
                    ctx_part(attn_part())
                else:
                    pending_epilogue = epi

            if pending_epilogue is not None:
                pending_epilogue()

    nc.compile()
    return nc


def _get_compiled():
    global _COMPILED
    if _COMPILED is None:
        _COMPILED = _build()
    return _COMPILED


def _prep_inputs(dec_hidden, enc_outputs, src_mask, attn_W, attn_b, v_W):
    dec_hidden = np.asarray(dec_hidden, dtype=np.float32)
    enc_outputs = np.asarray(enc_outputs, dtype=np.float32)
    src_mask = np.asarray(src_mask)
    attn_W = np.asarray(attn_W, dtype=np.float32)
    attn_b_np = np.asarray(attn_b, dtype=np.float32)
    v_W = np.asarray(v_W, dtype=np.float32)

    bf = ml_dtypes.bfloat16
    w_dec = np.ascontiguousarray(attn_W[:D]).astype(bf)
    w_enc = np.ascontiguousarray(attn_W[D:]).astype(bf)
    # Partition-major packing for the small constants: [128, x] tiles whose
    # DMA is contiguous per partition (strided small-element loads are
    # descriptor bombs that clog the SDMA engines at startup).
    v_np = np.ascontiguousarray(v_W[:, 0].reshape(DT, 128).T).astype(bf)
    attnb_packed = np.ascontiguousarray(attn_b_np.reshape(DT, 128).T)
    add_mask = np.where(src_mask, 0.0, -10000.0).astype(np.float32)
    add_mask = add_mask.reshape(B, NCHB, 128)

    enc_b16 = enc_outputs.astype(bf)                         # [B, S, E] bf16
    encT = np.ascontiguousarray(enc_b16.transpose(0, 2, 1))  # [B, E, S] bf16

    in_maps = []
    for i in range(N_CORES):
        lo, hi = i * NB, (i + 1) * NB
        in_maps.append(
            {
                "encT_bf": encT[lo:hi],
                "enc_bf": enc_b16[lo:hi],
                "w_enc": w_enc,
                "w_dec": w_dec,
                # dec_hT packed: [128 p, KT, NB] with k = kt*128 + p
                "dec_hT": np.ascontiguousarray(
                    dec_hidden[lo:hi].T.reshape(KT, 128, NB).transpose(1, 0, 2)
                ).astype(bf),
                "attn_b": attnb_packed,
                "v_in": v_np,
                # maskT packed: [128 p, (b, chunk)] with s = chunk*128 + p
                "maskT": np.ascontiguousarray(
                    add_mask[lo:hi].transpose(2, 0, 1).reshape(128, NB * NCHB)
                ),
            }
        )
    return in_maps


def run(inputs, trace=False, tmpdir=None):
    """Run the kernel; returns ((context, attn_w), BassKernelResults)."""
    nc = _get_compiled()
    in_maps = _prep_inputs(**inputs)
    res = run_bass_kernel_spmd(
        nc, in_maps, core_ids=list(range(N_CORES)), trace=trace, tmpdir=tmpdir
    )
    ctx = np.concatenate([res.results[i]["out_ctx"] for i in range(N_CORES)], axis=0)
    attn = np.concatenate([res.results[i]["out_attn"] for i in range(N_CORES)], axis=0)
    return (ctx, attn), res


def kernel(**inputs):
    (ctx, attn), _ = run(inputs, trace=False)
    return ctx, attn


if __name__ == "__main__":
    try:
        import reference
    except ImportError:
        print("reference.py not available; import kernel and call kernel(**inputs)")
    else:
        inp = {k: np.asarray(v) for k, v in reference.setup_inputs().items()}
        ctx, attn = kernel(**inp)
        print(ctx.shape, attn.shape)


# revision 27
# speedup vs baseline: 1.0122x; 1.0122x over previous
"""Bahdanau attention (optimized) Trainium2 kernel.

Problem (hardcoded shapes): B=32, S=2048, ENC=DEC=1024.
    dec_proj = dec_hidden @ W_dec + b                  [B, DEC]
    projT    = W_enc.T @ enc_outputs[b].T              [DEC, S]  (per batch)
    energyT  = tanh(projT + dec_proj[b][:, None])      [DEC, S]
    scores   = v.T @ energyT (+ mask)                  [S]
    attn     = softmax(scores)                         (scores in ~[-4, 4]: plain exp is safe)
    context  = attn @ enc_outputs[b]                   [ENC]

Sharding: data-parallel over batch, 4 batches per core on 8 NeuronCores.
Each core holds the full (small) weights.

Host-side prep (layout only): enc is fed twice in bf16 — once pre-transposed
to [DEC-on-partitions] for the projection matmul (the PE contracts along
partitions, and the natural layout has S on partitions), and once in natural
layout for the context matmul.  All matmuls run in bf16 with fp32 PSUM
accumulation (measured end-to-end rel. error ~3.3e-3); softmax bookkeeping
stays fp32.

Per 512-wide sequence block: 64 projection matmuls (N=512, stationary=W tile)
stream back-to-back at the PE N=512 roofline (~216ns spacing); tanh with
per-partition bias on ACT; the 8 scores matmuls (stationary = 1-column v) are
batched per block; scores row -> columns via tiny K=1 float32r matmuls; exp
on ACT with additive-mask bias.  The context accumulation runs on the
otherwise-idle DVE as a fused scalar_tensor_tensor chain (acc += en * w per
128-chunk) with a final PE partition-reduce; the last block of the last
batch goes through the PE so the serial DVE chain never becomes the kernel
tail.  Block tails and batch epilogues are software-pipelined into the next
block's projection j-loop so the PE stream stays gapless; weights and the
first enc tiles arrive as k-sliced DMAs spread over the three DMA rings
(scalar/sync HWDGE + gpsimd SWDGE) to cut the startup ramp.
"""

import os
import sys

for _p in ("/opt/trn_rl_repo",):
    if os.path.isdir(_p) and _p not in sys.path:
        sys.path.append(_p)

import numpy as np
import ml_dtypes

import concourse.bass as bass
import concourse.mybir as mybir
import concourse.tile as tile
from concourse import bacc
from concourse.bass_utils import run_bass_kernel_spmd
from concourse.masks import make_identity

BF16 = mybir.dt.bfloat16
F32 = mybir.dt.float32
F32R = mybir.dt.float32r

N_CORES = 8
B, S, E, D = 32, 2048, 1024, 1024
NB = B // N_CORES          # batches per core = 4
SB = 512                   # sequence block
NBLK = S // SB             # 4 blocks per batch
NCH = SB // 128            # 4 chunks of 128 per block
NCHB = S // 128            # 16 chunks per batch
KT = E // 128              # 8 contraction tiles
DT = D // 128              # 8 d tiles

_COMPILED = None


def _build():
    nc = bacc.Bacc(None, target_bir_lowering=False, debug=False)

    encT_bf = nc.dram_tensor("encT_bf", [NB, E, S], BF16, kind="ExternalInput").ap()
    enc_bf = nc.dram_tensor("enc_bf", [NB, S, E], BF16, kind="ExternalInput").ap()
    w_enc = nc.dram_tensor("w_enc", [E, D], BF16, kind="ExternalInput").ap()
    w_dec = nc.dram_tensor("w_dec", [D, D], BF16, kind="ExternalInput").ap()
    dec_hT = nc.dram_tensor("dec_hT", [128, KT, NB], BF16, kind="ExternalInput").ap()
    attn_b = nc.dram_tensor("attn_b", [128, DT], F32, kind="ExternalInput").ap()
    v_in = nc.dram_tensor("v_in", [128, DT], BF16, kind="ExternalInput").ap()
    maskT = nc.dram_tensor("maskT", [128, NB * NCHB], F32, kind="ExternalInput").ap()

    out_ctx = nc.dram_tensor("out_ctx", [NB, E], F32, kind="ExternalOutput").ap()
    out_attn = nc.dram_tensor("out_attn", [NB, S], F32, kind="ExternalOutput").ap()

    with tile.TileContext(nc) as tc:
        with (
            tc.tile_pool(name="const", bufs=1) as const_pool,
            tc.tile_pool(name="et", bufs=2) as et_pool,
            tc.tile_pool(name="en", bufs=2) as en_pool,
            tc.tile_pool(name="energy", bufs=2) as energy_pool,
            tc.tile_pool(name="expw", bufs=2) as expw_pool,
            tc.tile_pool(name="acc", bufs=2) as acc_pool,
            tc.tile_pool(name="small", bufs=4) as small_pool,
            tc.tile_pool(name="pproj", bufs=2, space="PSUM") as proj_psum,
            tc.tile_pool(name="psmall", bufs=2, space="PSUM") as small_psum,
            tc.tile_pool(name="pctxr", bufs=2, space="PSUM") as ctxr_psum,
        ):
            # ---- resident constants -------------------------------------
            # Queue split: the critical first bytes (w_enc on scalar, et0 on
            # sync) get their rings to themselves; w_dec streams through
            # SWDGE as 8 k-slices so the interleaved dec_proj matmuls can
            # start on slice 0.  Tiny consts go first (KBs).
            dech_sb = const_pool.tile([128, KT, NB], BF16)
            nc.scalar.dma_start(dech_sb[:], dec_hT[:])
            attnb_sb = const_pool.tile([128, DT], F32)
            nc.scalar.dma_start(attnb_sb[:], attn_b[:])
            v_sb = const_pool.tile([128, DT], BF16)
            nc.scalar.dma_start(v_sb[:], v_in[:])
            mask_sb = const_pool.tile([128, NB * NCHB], F32)
            nc.scalar.dma_start(mask_sb[:], maskT[:])
            w_enc_sb = const_pool.tile([128, KT, D], BF16)
            w_enc_r = w_enc.rearrange("(k p) d -> p k d", p=128)
            for k in range(KT):
                nc.scalar.dma_start(w_enc_sb[:, k, :], w_enc_r[:, k, :])
            w_dec_sb = const_pool.tile([128, KT, D], BF16)
            w_dec_r = w_dec.rearrange("(k p) d -> p k d", p=128)
            for k in range(KT):
                nc.gpsimd.dma_start(w_dec_sb[:, k, :], w_dec_r[:, k, :])

            one_11 = const_pool.tile([1, 1], F32)
            nc.gpsimd.memset(one_11[:], 1.0)
            one_2f = const_pool.tile([1, 2], F32)
            nc.gpsimd.memset(one_2f[:], 1.0)
            one_2r = const_pool.tile([1, 2], F32R)
            nc.vector.tensor_copy(one_2r[:], one_2f[:])
            ones_row = const_pool.tile([1, 128], F32)
            nc.gpsimd.memset(ones_row[:], 1.0)
            ones_2 = const_pool.tile([128, 2], F32)
            nc.gpsimd.memset(ones_2[:], 1.0)
            ones_col_bf = const_pool.tile([128, 1], BF16)
            nc.gpsimd.memset(ones_col_bf[:], 1.0)
            id_sb = const_pool.tile([128, 128], BF16)
            make_identity(nc, id_sb[:])

            # ---- dec_projT[d, b] = (dec_hidden @ W_dec + b).T ------------
            # Computed as dec_proj rows [NB, D] with wide (N=512) matmuls,
            # then 8 tiny PE transposes to columns.  Emitted inside the first
            # block (after proj group j=0) so the first projection matmuls
            # don't wait for w_dec: tanh j stalls briefly, which the
            # block-end scores batch absorbs.
            dp_sb = const_pool.tile([128, DT, NB], F32)

            def emit_dec_proj():
                dpp = ctxr_psum.tile([NB, D], F32, tag="pcr")
                for k in range(KT):
                    for h in range(2):
                        nc.tensor.matmul(
                            dpp[:, h * 512:(h + 1) * 512],
                            dech_sb[:, k, :],
                            w_dec_sb[:, k, h * 512:(h + 1) * 512],
                            start=(k == 0),
                            stop=(k == KT - 1),
                            skip_group_check=True,
                        )
                dpr = small_pool.tile([NB, D], BF16, tag="dpr")
                nc.scalar.activation(
                    dpr[:], dpp[:], mybir.ActivationFunctionType.Copy
                )
                for j in range(DT):
                    pj = small_psum.tile([128, NB], BF16, tag="sp")
                    nc.tensor.transpose(
                        pj[:], dpr[:, j * 128:(j + 1) * 128], id_sb[:NB, :NB]
                    )
                    nc.scalar.activation(
                        dp_sb[:, j, :],
                        pj[:],
                        mybir.ActivationFunctionType.Identity,
                        bias=attnb_sb[:, j:j + 1],
                    )

            emit_dec_proj()

            # ---- main loop ----------------------------------------------
            # Software pipelining: each block's tail (scores matmuls, the
            # scores-row transpose, exp, and the context accumulation) is
            # emitted inside the NEXT block's projection j-loop, so the PE
            # never waits on the ACT/DVE hops in the tail chain.  Batch
            # epilogues are likewise deferred into the next batch's second
            # block.
            pending_epilogue = None
            pending_tail = None
            for b in range(NB):
                expw_all = expw_pool.tile([128, NCHB], F32)
                state = {"acc": None, "pcr": None, "sc": None}

                for blk in range(NBLK):
                    t0 = blk * NCH
                    et = et_pool.tile([128, KT, SB], BF16)
                    et_src = encT_bf[b].rearrange("(k p) s -> p k s", p=128)[
                        :, :, blk * SB:(blk + 1) * SB
                    ]
                    nc.sync.dma_start(et[:, :KT // 2, :], et_src[:, :KT // 2, :])
                    nc.sync.dma_start(et[:, KT // 2:, :], et_src[:, KT // 2:, :])
                    en = en_pool.tile([128, NCH, E], BF16)
                    nc.scalar.dma_start(
                        en[:],
                        enc_bf[b, blk * SB:(blk + 1) * SB, :].rearrange(
                            "(c p) e -> p c e", p=128
                        ),
                    )

                    egy = energy_pool.tile([128, DT, SB], BF16)
                    for j in range(DT):
                        pp = proj_psum.tile([128, SB], F32)
                        for k in range(KT):
                            nc.tensor.matmul(
                                pp[:],
                                w_enc_sb[:, k, j * 128:(j + 1) * 128],
                                et[:, k, :],
                                start=(k == 0),
                                stop=(k == KT - 1),
                            )
                        if pending_tail is not None:
                            if j == 0:
                                pending_tail[0]()
                            elif j == 1:
                                pending_tail[1]()
                                pending_tail = None
                        if blk == 1 and j == 3 and pending_epilogue is not None:
                            pending_epilogue()
                            pending_epilogue = None
                        nc.scalar.activation(
                            egy[:, j, :],
                            pp[:],
                            mybir.ActivationFunctionType.Tanh,
                            bias=dp_sb[:, j, b:b + 1],
                        )

                    def tail1(b=b, egy=egy, state=state):
                        # scores: batched v-matmuls (stationary = 1-col v)
                        psc = small_psum.tile([1, SB], F32, tag="sp")
                        for j in range(DT):
                            nc.tensor.matmul(
                                psc[:],
                                v_sb[:, j:j + 1],
                                egy[:, j, :],
                                start=(j == 0),
                                stop=(j == DT - 1),
                                skip_group_check=True,
                            )
                        sc_sb = small_pool.tile([1, SB], F32R, tag="scsb")
                        nc.vector.tensor_copy(sc_sb[:], psc[:])
                        state["sc"] = sc_sb

                    def tail2(b=b, blk=blk, t0=t0, en=en, state=state,
                              expw_all=expw_all):
                        sc_sb = state["sc"]
                        # scores row -> 4 columns of 128 (K=1 f32r matmuls)
                        pt = small_psum.tile([128, 2 * NCH], F32, tag="sp")
                        for c in range(NCH):
                            nc.tensor.matmul(
                                pt[:, 2 * c:2 * c + 2],
                                sc_sb[:, c * 128:(c + 1) * 128],
                                one_2r[:],
                                start=True,
                                stop=True,
                                skip_group_check=True,
                            )
                        for c in range(NCH):
                            t = t0 + c
                            nc.scalar.activation(
                                expw_all[:, t:t + 1],
                                pt[:, 2 * c:2 * c + 1],
                                mybir.ActivationFunctionType.Exp,
                                bias=mask_sb[:, b * NCHB + t:b * NCHB + t + 1],
                            )
                        # context: blocks 0..2 (and all blocks of non-final
                        # batches) accumulate on the otherwise-idle DVE:
                        #   acc[p,e] (+)= en[p,e] * expw[p]  per chunk.
                        # The final block of the final batch goes through the
                        # PE instead (the serial DVE chain would be the
                        # kernel tail), merged with the acc partition-reduce.
                        if blk < NBLK - 1 or b < NB - 1:
                            acc = state["acc"]
                            for c in range(NCH):
                                t = t0 + c
                                last = t == (
                                    4 * NCH - NCH - 1 if b == NB - 1 else NCHB - 1
                                )
                                nacc = acc_pool.tile(
                                    [128, E], BF16 if last else F32,
                                    tag="accb" if last else "acc",
                                )
                                if acc is None:
                                    nc.vector.tensor_scalar_mul(
                                        nacc[:], en[:, c, :], expw_all[:, t:t + 1]
                                    )
                                else:
                                    nc.vector.scalar_tensor_tensor(
                                        nacc[:], en[:, c, :],
                                        expw_all[:, t:t + 1], acc[:],
                                        mybir.AluOpType.mult,
                                        mybir.AluOpType.add,
                                    )
                                acc = nacc
                            state["acc"] = acc
                        else:
                            acc = state["acc"]
                            pcr = ctxr_psum.tile([1, E], F32, tag="pcr")
                            for h in range(2):
                                nc.tensor.matmul(
                                    pcr[:, h * 512:(h + 1) * 512],
                                    ones_col_bf[:],
                                    acc[:, h * 512:(h + 1) * 512],
                                    start=True,
                                    stop=False,
                                    skip_group_check=True,
                                )
                            ewb = small_pool.tile([128, NCH], BF16, tag="ewb")
                            nc.vector.tensor_copy(
                                ewb[:], expw_all[:, t0:t0 + NCH]
                            )
                            for c in range(NCH):
                                for h in range(2):
                                    nc.tensor.matmul(
                                        pcr[:, h * 512:(h + 1) * 512],
                                        ewb[:, c:c + 1],
                                        en[:, c, h * 512:(h + 1) * 512],
                                        start=False,
                                        stop=(c == NCH - 1),
                                        skip_group_check=True,
                                    )
                            state["pcr"] = pcr

                    if b == NB - 1 and blk == NBLK - 1:
                        tail1()
                        tail2()
                    else:
                        pending_tail = (tail1, tail2)

                # ---- batch epilogue (deferred into the next batch) ------
                def make_epilogue(b=b, state=state, expw_all=expw_all):
                    def attn_part():
                        # Z = sum(expw): free-reduce on DVE, partition-reduce
                        # + broadcast via tiny in-stream PE matmuls (gpsimd
                        # custom ops would thrash Q7 libraries, ~7us each).
                        zc = small_pool.tile([128, 1], F32, tag="zc")
                        nc.vector.tensor_reduce(
                            zc[:], expw_all[:], mybir.AxisListType.X,
                            mybir.AluOpType.add,
                        )
                        pzz = small_psum.tile([1, 2], F32, tag="sp")
                        nc.tensor.matmul(pzz[:], zc[:], ones_2[:],
                                         start=True, stop=True,
                                         skip_group_check=True)
                        z_sb = small_pool.tile([1, 1], F32, tag="z")
                        nc.vector.tensor_copy(z_sb[:], pzz[:, 0:1])
                        rz = small_pool.tile([1, 1], F32, tag="rz")
                        nc.vector.reciprocal(rz[:], z_sb[:])
                        # transpose the unnormalized expw in parallel with
                        # the Z chain; 1/Z is applied on the PSUM-evacuating
                        # ACT copy via a 16-partition broadcast.
                        prz = small_psum.tile([16, 1], F32, tag="sp")
                        nc.tensor.matmul(prz[:], ones_row[:, :16], rz[:],
                                         start=True, stop=True,
                                         skip_group_check=True)
                        rz16 = small_pool.tile([16, 1], F32, tag="rzc")
                        nc.vector.tensor_copy(rz16[:], prz[:])

                        ewb2 = small_pool.tile([128, NCHB], BF16, tag="af")
                        nc.vector.tensor_copy(ewb2[:], expw_all[:])
                        pat = small_psum.tile([16, 128], BF16, tag="sp")
                        nc.tensor.transpose(pat[:], ewb2[:], id_sb[:])
                        at_sb = small_pool.tile([16, 128], F32, tag="atsb")
                        nc.scalar.activation(
                            at_sb[:],
                            pat[:],
                            mybir.ActivationFunctionType.Copy,
                            scale=rz16[:],
                        )
                        nc.sync.dma_start(
                            out_attn[b].rearrange("(t s) -> t s", s=128),
                            at_sb[:],
                        )
                        return rz

                    def ctx_part(rz):
                        if state["pcr"] is None:
                            acc = state["acc"]
                            pcr_l = ctxr_psum.tile([1, E], F32, tag="pcr")
                            for h in range(2):
                                nc.tensor.matmul(
                                    pcr_l[:, h * 512:(h + 1) * 512],
                                    ones_col_bf[:],
                                    acc[:, h * 512:(h + 1) * 512],
                                    start=True,
                                    stop=True,
                                    skip_group_check=True,
                                )
                        else:
                            pcr_l = state["pcr"]
                        ctx_sb = small_pool.tile([1, E], F32, tag="ctx")
                        nc.scalar.activation(
                            ctx_sb[:],
                            pcr_l[:],
                            mybir.ActivationFunctionType.Copy,
                            scale=rz[:],
                        )
                        nc.scalar.dma_start(out_ctx[b], ctx_sb[:])

                    def epi():
                        ctx_part(attn_part())
                    return epi, attn_part, ctx_part

                epi, attn_part, ctx_part = make_epilogue()
                if b == NB - 1:
                    # Final batch: tails were emitted inline; finish now.
                    ctx_part(attn_part())
                else:
                    pending_epilogue = epi

            if pending_epilogue is not None:
                pending_epilogue()

    nc.compile()
    return nc


def _get_compiled():
    global _COMPILED
    if _COMPILED is None:
        _COMPILED = _build()
    return _COMPILED


def _prep_inputs(dec_hidden, enc_outputs, src_mask, attn_W, attn_b, v_W):
    dec_hidden = np.asarray(dec_hidden, dtype=np.float32)
    enc_outputs = np.asarray(enc_outputs, dtype=np.float32)
    src_mask = np.asarray(src_mask)
    attn_W = np.asarray(attn_W, dtype=np.float32)
    attn_b_np = np.asarray(attn_b, dtype=np.float32)
    v_W = np.asarray(v_W, dtype=np.float32)

    bf = ml_dtypes.bfloat16
    w_dec = np.ascontiguousarray(attn_W[:D]).astype(bf)
    w_enc = np.ascontiguousarray(attn_W[D:]).astype(bf)
    # Partition-major packing for the small constants: [128, x] tiles whose
    # DMA is contiguous per partition (strided small-element loads are
    # descriptor bombs that clog the SDMA engines at startup).
    v_np = np.ascontiguousarray(v_W[:, 0].reshape(DT, 128).T).astype(bf)
    attnb_packed = np.ascontiguousarray(attn_b_np.reshape(DT, 128).T)
    add_mask = np.where(src_mask, 0.0, -10000.0).astype(np.float32)
    add_mask = add_mask.reshape(B, NCHB, 128)

    enc_b16 = enc_outputs.astype(bf)                         # [B, S, E] bf16
    encT = np.ascontiguousarray(enc_b16.transpose(0, 2, 1))  # [B, E, S] bf16

    in_maps = []
    for i in range(N_CORES):
        lo, hi = i * NB, (i + 1) * NB
        in_maps.append(
            {
                "encT_bf": encT[lo:hi],
                "enc_bf": enc_b16[lo:hi],
                "w_enc": w_enc,
                "w_dec": w_dec,
                # dec_hT packed: [128 p, KT, NB] with k = kt*128 + p
                "dec_hT": np.ascontiguousarray(
                    dec_hidden[lo:hi].T.reshape(KT, 128, NB).transpose(1, 0, 2)
                ).astype(bf),
                "attn_b": attnb_packed,
                "v_in": v_np,
                # maskT packed: [128 p, (b, chunk)] with s = chunk*128 + p
                "maskT": np.ascontiguousarray(
                    add_mask[lo:hi].transpose(2, 0, 1).reshape(128, NB * NCHB)
                ),
            }
        )
    return in_maps


def run(inputs, trace=False, tmpdir=None):
    """Run the kernel; returns ((context, attn_w), BassKernelResults)."""
    nc = _get_compiled()
    in_maps = _prep_inputs(**inputs)
    res = run_bass_kernel_spmd(
        nc, in_maps, core_ids=list(range(N_CORES)), trace=trace, tmpdir=tmpdir
    )
    ctx = np.concatenate([res.results[i]["out_ctx"] for i in range(N_CORES)], axis=0)
    attn = np.concatenate([res.results[i]["out_attn"] for i in range(N_CORES)], axis=0)
    return (ctx, attn), res


def kernel(**inputs):
    (ctx, attn), _ = run(inputs, trace=False)
    return ctx, attn


if __name__ == "__main__":
    try:
        import reference
    except ImportError:
        print("reference.py not available; import kernel and call kernel(**inputs)")
    else:
        inp = {k: np.asarray(v) for k, v in reference.setup_inputs().items()}
        ctx, attn = kernel(**inp)
        print(ctx.shape, attn.shape)
